# revision 30
# baseline (speedup 1.0000x reference)
"""3-layer GCN encoder on 8 TRN2 NeuronCores (Bass/Tile).

Strategy (see spec sharding_hint): nodes degree-rank-dealt across 8 cores
(12500 + 44 pad rows each). Per layer, per core:
- g_nm = (x'^T block)^T @ W per 128-node block on PE (node-major direct),
  batched PSUM->SBUF copies alternating Act/DVE engines;
- two safe-size AllGathers (A/B shard halves, each with a 128-row zero
  head) into a full fp32 gather table;
- bulk dma_gather of per-edge messages over two int16 windows (signed
  offsets from shifted bases), spread across 4 SWDGE queues (the single-
  queue descriptor pipeline is ~10x slower); window assignment is forced
  by reach with the overlap region flex-balanced per destination;
- padded-CSR degree-sorted blocks, run-merged DVE segment reduces,
  per-window alignment re-gather (w1's overlaps w2's message gathers),
  chunked add/relu/scale postproc pipelined with next-layer transposes;
- self-loop term added on-chip (u += g_nm), biases-zero fast path
  (dis scaling commutes with relu).
All heavy arithmetic on device; host does index preprocessing only.
"""

import sys

sys.path.insert(0, "/opt/trn_rl_repo")

import numpy as np

from concourse import bass, bacc, mybir, tile
import concourse.bass_utils as bass_utils
from concourse.masks import make_identity

# ---------------- problem constants (hardcoded per harness contract) -------
N = 100000
E = 1600000
F = 64  # feature width used everywhere (W3 zero-padded 32->64)
OC = 32
NCORES = 8
KB = 98  # blocks per core
SH = KB * 128  # 12544 rows per core shard
NPC = 12500  # real nodes per core

# table layout: two safe-size AllGathers (A = p<64 halves, B = p>=64),
# each shard half led by a 128-row zero head:
#   shard = [128 z | 6272 gA | 128 z | 6272 gB];  table = [A: 8*6400 | B: 8*6400]
HB = 6400  # half-shard rows incl zero head
SH2 = 2 * HB  # 12800
A_SIZE = NCORES * HB  # 51200
T_ROWS = 2 * A_SIZE  # 102400
BASE_1 = 32768  # w1 reach [0, 65535]: all A + B head (flex)
BASE_2 = T_ROWS - 32768  # 69632; w2 reach [36864, 102399]
W1_MAX = 65536  # trow < this -> w1-capable
ZREL_1 = 6 * HB - BASE_1  # core6 A-zero head (5632)
ZREL_2 = A_SIZE + 3 * HB - BASE_2  # core3 B-zero head (768)

GMAX_COLS = 40  # max slot-columns per gather group

f32 = mybir.dt.float32
i16 = mybir.dt.int16

_CACHE = {}


# ============================ host preprocessing ===========================

def _wrap_idx(vals: np.ndarray) -> np.ndarray:
    """[n] int -> [128, n//16] int16 (wrapped in 16 partitions, replicated x8)."""
    n = len(vals)
    assert n % 16 == 0
    a = vals.reshape(n // 16, 16).T.astype(np.int16)
    return np.tile(a, (8, 1))


def _preprocess(edge_index: np.ndarray):
    src = np.asarray(edge_index[0], dtype=np.int64)
    dst = np.asarray(edge_index[1], dtype=np.int64)
    deg = np.bincount(dst, minlength=N).astype(np.int64) + 1

    # global degree-rank deal: rank r -> core r%8, in-core rank j=r//8
    order = np.argsort(-deg, kind="stable")  # rank -> node
    node_core = np.empty(N, np.int32)
    node_j = np.empty(N, np.int32)
    ranks = np.arange(N)
    node_core[order] = (ranks % NCORES).astype(np.int32)
    node_j[order] = (ranks // NCORES).astype(np.int32)

    # in-core coords: j -> (k=j//128, p=j%128); table row:
    #   p<64:  c*HB + 128 + p*KB + k              (A region)
    #   p>=64: A_SIZE + c*HB + 128 + (p-64)*KB + k (B region)
    k_of = node_j // 128
    p_of = node_j % 128
    cc = node_core.astype(np.int64)
    trow = np.where(
        p_of < 64,
        cc * HB + 128 + p_of * KB + k_of,
        A_SIZE + cc * HB + 128 + (p_of - 64) * KB + k_of,
    ).astype(np.int64)

    # incoming edge lists grouped by dst
    eorder = np.argsort(dst, kind="stable")
    src_sorted = src[eorder]
    dst_sorted = dst[eorder]
    counts = np.bincount(dst, minlength=N)
    starts = np.zeros(N + 1, np.int64)
    np.cumsum(counts, out=starts[1:])

    # window per edge: A sources forced w1, B>=65536 forced w2,
    # B head [A_SIZE, 65536) flexible -> balanced per dst
    all_rows = trow[src_sorted]
    w_of_row = np.full(E, -1, np.int8)
    w_of_row[all_rows < A_SIZE] = 0
    w_of_row[all_rows >= W1_MAX] = 1
    flex = w_of_row == -1
    a_c = np.bincount(dst_sorted[w_of_row == 0], minlength=N)
    b_c = np.bincount(dst_sorted[w_of_row == 1], minlength=N)
    f_c = np.bincount(dst_sorted[flex], minlength=N)
    x_to_w1 = np.clip((b_c + f_c - a_c + 1) // 2, 0, f_c)
    idx_flex = np.nonzero(flex)[0]
    dsts_f = dst_sorted[idx_flex]
    seg_start = starts[dsts_f]  # not flex-seg start; compute flex rank:
    # rank of each flex edge within its dst's flex group (dst-sorted order)
    uniq, grp_first = np.unique(dsts_f, return_index=True)
    rank = np.arange(len(dsts_f)) - np.repeat(
        grp_first, np.diff(np.r_[grp_first, len(dsts_f)])
    )
    w_of_row[idx_flex] = (rank >= x_to_w1[dsts_f]).astype(np.int8)

    # per-(node, window) counts (self-loop handled on-chip: u += g_nm)
    cnt_w = np.zeros((N, 2), np.int32)
    np.add.at(cnt_w, (dst_sorted, w_of_row.astype(np.int64)), 1)

    cores = []
    # shared block schedule D[k][w]: max over cores of sorted per-window counts
    percore = []
    for c in range(NCORES):
        nodes_c = order[c::NCORES]  # in-core rank j -> node
        percore.append(nodes_c)
    D = np.zeros((2, KB), np.int32)
    pi_w = []  # [w][c] -> permutation over in-core slots (j indices incl pads)
    for w in range(2):
        perms = []
        for c in range(NCORES):
            nodes_c = percore[c]
            cw = np.zeros(SH, np.int32)
            cw[: len(nodes_c)] = cnt_w[nodes_c, w]
            perm = np.argsort(-cw, kind="stable")  # slot j' -> in-core rank j
            perms.append(perm)
            sorted_c = cw[perm]
            blk_max = sorted_c.reshape(KB, 128)[:, 0]
            D[w] = np.maximum(D[w], blk_max)
        pi_w.append(perms)
    D = np.maximum(D, 1)

    # group packing: blocks -> groups with <= GMAX_COLS slot columns
    groups = [[], []]  # [w] -> list of list-of-block-ids
    for w in range(2):
        cur, cur_cols = [], 0
        for kb in range(KB):
            d = int(D[w][kb])
            if cur and cur_cols + d > GMAX_COLS:
                groups[w].append(cur)
                cur, cur_cols = [], 0
            cur.append(kb)
            cur_cols += d
        if cur:
            groups[w].append(cur)

    zrel = (ZREL_1, ZREL_2)
    base_w = (BASE_1, BASE_2)

    # build per-core idx blobs + align idx + per-core arrays
    idx_blob = [[None] * NCORES, [None] * NCORES]
    al_idx = [[None] * NCORES, [None] * NCORES]
    deg_arr = [None] * NCORES
    for c in range(NCORES):
        nodes_c = percore[c]
        nc_nodes = len(nodes_c)
        # per (in-core rank, window) row lists
        for w in range(2):
            perm = pi_w[w][c]
            cols_parts = []
            for g in groups[w]:
                gcols = []
                for kb in g:
                    d = int(D[w][kb])
                    block = perm[kb * 128 : (kb + 1) * 128]  # 128 in-core ranks
                    # rows matrix [128, d] filled with zrel
                    m = np.full((128, d), zrel[w], np.int64)
                    for p in range(128):
                        j = block[p]
                        if j >= nc_nodes:
                            continue
                        node = nodes_c[j]
                        rows = all_rows[starts[node] : starts[node + 1]]
                        rows = rows[w_of_row[starts[node] : starts[node + 1]] == w]
                        if len(rows):
                            m[p, : len(rows)] = rows - base_w[w]
                    gcols.append(m)
                gm = np.concatenate(gcols, axis=1)  # [128, cols_g]
                gm = np.concatenate(
                    [gm, np.full((128, 1), zrel[w], np.int64)], axis=1
                )  # sentinel col
                # positions: col-major: pos = col*128 + p
                cols_parts.append(_wrap_idx(gm.T.ravel()))
            idx_blob[w][c] = np.concatenate(cols_parts, axis=1)

            # align gather: master pos (k*128+p) -> u_w dram row p'*98+k'
            inv = np.empty(SH, np.int64)  # in-core rank j -> slot index in pi_w
            inv[perm] = np.arange(SH)
            jj = np.arange(SH)  # master position index = k*128 + p
            kk = jj // 128
            pp = jj % 128
            j_of_pos = kk * 128 + pp  # in-core rank at master (p, k) = k*128+p
            sw = inv[j_of_pos]
            al = (sw % 128) * KB + sw // 128  # u_w dram row
            al_idx[w][c] = _wrap_idx(al)

        dg = np.full((128, KB), 1e30, np.float32)
        jj = np.arange(nc_nodes)
        dg[jj % 128, jj // 128] = deg[nodes_c].astype(np.float32)
        deg_arr[c] = dg

    maxg = max(
        sum(int(D[w][kb]) for kb in g) + 1 for w in range(2) for g in groups[w]
    )
    meta = dict(
        D=D,
        groups=groups,
        maxg=maxg,
        percore=percore,
        idx_blob=idx_blob,
        al_idx=al_idx,
        deg_arr=deg_arr,
    )
    return meta


# ============================ device kernel ================================

def _build(meta, with_bias: bool, reps: int = 1):
    import os

    stop = os.environ.get("KBUILD_STOP", "full")  # ag|gather|reduce|align|full
    nlayers = int(os.environ.get("KBUILD_NLAYERS", "3"))
    max_groups = int(os.environ.get("KBUILD_MAX_GROUPS", "9999"))
    skip_ag = os.environ.get("KBUILD_SKIP_AG", "") == "1"
    nwin = int(os.environ.get("KBUILD_NWIN", "2"))
    D, groups = meta["D"], meta["groups"]
    nq = int(os.environ.get("KBUILD_NQ", "4"))
    nc = bacc.Bacc("TRN2", target_bir_lowering=False, debug=False,
                   num_devices=NCORES, num_swdge_queues=nq)

    x_in = nc.dram_tensor("x_nm", [128, KB * F], f32, kind="ExternalInput")
    deg_in = nc.dram_tensor("deg", [128, KB], f32, kind="ExternalInput")
    w_in = [
        nc.dram_tensor(f"w{l}", [F, F], f32, kind="ExternalInput") for l in (1, 2, 3)
    ]
    b_in = [
        nc.dram_tensor(f"b{l}", [1, F], f32, kind="ExternalInput") for l in (1, 2, 3)
    ]
    idx_in = [
        nc.dram_tensor(
            f"idxw{w + 1}", list(meta["idx_blob"][w][0].shape), i16, kind="ExternalInput"
        )
        for w in range(2)
    ]
    al_in = [
        nc.dram_tensor(f"alw{w + 1}", [128, SH // 16], i16, kind="ExternalInput")
        for w in range(2)
    ]
    x_out = [
        nc.dram_tensor(f"x{l}o", [128, KB * F], f32, kind="ExternalOutput")
        for l in (1, 2, 3)
    ]

    with tile.TileContext(nc) as tc:
        with (
            tc.tile_pool(name="const", bufs=1) as cpool,
            tc.tile_pool(name="sbuf", bufs=2) as sb,
            tc.tile_pool(name="big", bufs=1) as bigp,
            tc.tile_pool(name="msgs", bufs=1) as msp,
            tc.tile_pool(name="psum_tr", bufs=4, space="PSUM") as ps_tr,
            tc.tile_pool(name="dram", bufs=1, space="DRAM") as dr,
        ):
            # ---- constants ----
            ident = cpool.tile([128, 128], f32)
            make_identity(nc, ident[:])
            w_sb = []
            for l in range(3):
                t = cpool.tile([F, F], f32, tag=f"w{l}")
                nc.sync.dma_start(out=t[:], in_=w_in[l][:, :])
                w_sb.append(t)
            b_sb = []
            if with_bias:
                for l in range(3):
                    t = cpool.tile([1, F], f32, tag=f"b{l}")
                    nc.sync.dma_start(out=t[:], in_=b_in[l][:, :])
                    b_sb.append(t)
            deg_sb = cpool.tile([128, KB], f32)
            nc.sync.dma_start(out=deg_sb[:], in_=deg_in[:, :])
            al_sb = []
            for w in range(2):
                t = cpool.tile([128, SH // 16], i16, tag=f"al{w}")
                nc.sync.dma_start(out=t[:], in_=al_in[w][:, :])
                al_sb.append(t)

            # dis = rsqrt(deg), dis2 = 1/deg  (Newton-refined)
            r0 = cpool.tile([128, KB], f32, tag="r0")
            nc.vector.reciprocal(out=r0[:], in_=deg_sb[:])
            tmp = cpool.tile([128, KB], f32, tag="rt")
            nc.vector.tensor_tensor(
                out=tmp[:], in0=deg_sb[:], in1=r0[:], op=mybir.AluOpType.mult
            )
            nc.vector.tensor_scalar(
                out=tmp[:], in0=tmp[:], scalar1=-1.0, scalar2=2.0,
                op0=mybir.AluOpType.mult, op1=mybir.AluOpType.add,
            )
            dis2_sb = cpool.tile([128, KB], f32, tag="dis2")
            nc.vector.tensor_tensor(
                out=dis2_sb[:], in0=r0[:], in1=tmp[:], op=mybir.AluOpType.mult
            )
            dis_sb = cpool.tile([128, KB], f32, tag="dis")
            nc.scalar.sqrt(out=dis_sb[:], in_=dis2_sb[:])
            # one Newton step for sqrt: dis = 0.5*dis*(3 - deg*dis^2)
            s2 = cpool.tile([128, KB], f32, tag="s2")
            nc.vector.tensor_tensor(
                out=s2[:], in0=dis_sb[:], in1=dis_sb[:], op=mybir.AluOpType.mult
            )
            nc.vector.tensor_tensor(
                out=s2[:], in0=s2[:], in1=deg_sb[:], op=mybir.AluOpType.mult
            )
            nc.vector.tensor_scalar(
                out=s2[:], in0=s2[:], scalar1=-0.5, scalar2=1.5,
                op0=mybir.AluOpType.mult, op1=mybir.AluOpType.add,
            )
            nc.vector.tensor_tensor(
                out=dis_sb[:], in0=dis_sb[:], in1=s2[:], op=mybir.AluOpType.mult
            )

            # ---- DRAM scratch ----
            table = dr.tile([T_ROWS, F], f32)
            g_shard = dr.tile([SH2, F], f32)
            u_dram = [
                dr.tile([SH, F], f32, tag=f"u{w}", name=f"u_dram{w}") for w in range(2)
            ]

            # zero heads of both shard halves (AGs replicate them into every
            # core's table, giving each window an in-reach, dep-clean zero)
            ztile = cpool.tile([128, F], f32, tag="z")
            nc.vector.memset(ztile[:], 0.0)
            nc.sync.dma_start(out=g_shard[0:128, :], in_=ztile[:])
            nc.sync.dma_start(out=g_shard[HB : HB + 128, :], in_=ztile[:])

            # persistent activation x'^T
            xpT = bigp.tile([F, SH], f32, tag="xpT")
            qi = [0]  # round-robin SWDGE queue counter

            def next_q():
                q = qi[0] % nq
                qi[0] += 1
                return q

            def transpose_to_xpT(xp_sb, k_lo=0, k_hi=KB):
                """xp_sb [128, KB*F] node-major -> xpT [F, SH] feature-major.
                4 transposes share one PSUM tile -> 1 batched copy,
                alternating copy engines to pipeline PE -> {Act, DVE}."""
                for i, k0 in enumerate(range(k_lo, k_hi, 4)):
                    kn = min(4, k_hi - k0)
                    pt = ps_tr.tile([F, 4 * 128], f32, space="PSUM", tag="trF")
                    for j in range(kn):
                        k = k0 + j
                        nc.tensor.transpose(
                            out=pt[:, j * 128 : (j + 1) * 128],
                            in_=xp_sb[:, k * F : (k + 1) * F],
                            identity=ident[:],
                        )
                    dst = xpT[:, k0 * 128 : (k0 + kn) * 128]
                    if i % 2 == 0:
                        nc.scalar.copy(out=dst, in_=pt[:, : kn * 128])
                    else:
                        nc.vector.tensor_copy(out=dst, in_=pt[:, : kn * 128])

            # ---- layer-0 front: x' = dis * x ----
            # (big-buffer tag sharing to fit SBUF: xnm->gT, xp->uw, outb->gnm)
            x_nm = bigp.tile([128, KB * F], f32, tag="gT")
            nc.sync.dma_start(out=x_nm[:], in_=x_in[:, :])
            xp0 = bigp.tile([128, KB * F], f32, tag="uw")
            nc.vector.tensor_tensor(
                out=xp0[:].rearrange("p (k f) -> p k f", k=KB),
                in0=x_nm[:].rearrange("p (k f) -> p k f", k=KB),
                in1=dis_sb[:].unsqueeze(-1).broadcast_to([128, KB, F]),
                op=mybir.AluOpType.mult,
            )
            transpose_to_xpT(xp0)

            for _rep in range(reps):
                for l in range(nlayers):
                    # ---- A+B: g_nm block = (xpT_k)^T @ W (node-major direct) ----
                    g_nm = bigp.tile([128, KB * F], f32, tag="gnm")
                    for i, k0 in enumerate(range(0, KB, 4)):
                        kn = min(4, KB - k0)
                        pt = ps_tr.tile([128, 4 * F], f32, space="PSUM", tag="trB")
                        for j in range(kn):
                            k = k0 + j
                            nc.tensor.matmul(
                                out=pt[:, j * F : (j + 1) * F],
                                lhsT=xpT[:, k * 128 : (k + 1) * 128],
                                rhs=w_sb[l][:],
                                start=True,
                                stop=True,
                            )
                        dst = g_nm[:, k0 * F : (k0 + kn) * F]
                        if i % 2 == 0:
                            nc.scalar.copy(out=dst, in_=pt[:, : kn * F])
                        else:
                            nc.vector.tensor_copy(out=dst, in_=pt[:, : kn * F])
                    # halves: p<64 -> A half rows 128+p*KB+k, p>=64 -> B half
                    nc.sync.dma_start(
                        out=g_shard[128:HB, :].rearrange(
                            "(p k) f -> p (k f)", p=64
                        ),
                        in_=g_nm[0:64, :],
                    )
                    nc.sync.dma_start(
                        out=g_shard[HB + 128 : SH2, :].rearrange(
                            "(p k) f -> p (k f)", p=64
                        ),
                        in_=g_nm[64:128, :],
                    )
                    if not skip_ag:
                        nc.gpsimd.collective_compute(
                            "AllGather",
                            mybir.AluOpType.bypass,
                            replica_groups=[list(range(NCORES))],
                            ins=[g_shard[0:HB, :].opt()],
                            outs=[table[0:A_SIZE, :].opt()],
                        )
                        nc.gpsimd.collective_compute(
                            "AllGather",
                            mybir.AluOpType.bypass,
                            replica_groups=[list(range(NCORES))],
                            ins=[g_shard[HB:SH2, :].opt()],
                            outs=[table[A_SIZE:T_ROWS, :].opt()],
                        )

                    if stop == "ag":
                        dbg = sb.tile([128, F], f32, tag="dbg")
                        nc.sync.dma_start(out=dbg[:], in_=table[0:128, :])
                        nc.sync.dma_start(out=x_out[l][:, 0:F], in_=dbg[:])
                        continue

                    # ---- C: window gathers + segment reduces ----
                    u_al = []
                    for w in range(nwin):
                        u_w = bigp.tile([128, KB * F], f32, tag="uw")
                        # w1 reads A + B-head: declared range spans both AGs'
                        # outputs; w2 reads B only (incl core3 zero head)
                        in_ap = (
                            table[BASE_1:W1_MAX, :]
                            if w == 0
                            else table[BASE_2 : BASE_2 + 16384, :]
                        )
                        off8 = 0
                        for g in groups[w][:max_groups]:
                            gcols = sum(int(D[w][kb]) for kb in g) + 1
                            nidx = gcols * 128
                            q = next_q()
                            idx_sb = sb.tile([128, nidx // 16], i16, tag=f"idx{q}")
                            nc.sync.dma_start(
                                out=idx_sb[:],
                                in_=idx_in[w][:, off8 : off8 + nidx // 16],
                            )
                            msgs = msp.tile(
                                [128, meta["maxg"], F], f32, tag=f"msgs{q}"
                            )
                            nc.gpsimd.dma_gather(
                                out_ap=msgs[:, :gcols, :],
                                in_ap=in_ap,
                                idxs_ap=idx_sb[:],
                                num_idxs=nidx,
                                num_idxs_reg=nidx,
                                elem_size=F,
                                single_packet=False,
                                queue_num=q,
                            )
                            if stop == "gather":
                                nc.scalar.copy(
                                    out=u_w[:, 0:F],
                                    in_=msgs[:, 0, :],
                                )
                            else:
                                # merge consecutive equal-D blocks into one
                                # wide reduce: in [p, r, f, d] -> out [p, r*f]
                                runs = []
                                for kb in g:
                                    d = int(D[w][kb])
                                    if runs and runs[-1][2] == d:
                                        runs[-1][1] += 1
                                    else:
                                        runs.append([kb, 1, d])
                                loc = 0
                                for kb0, r, d in runs:
                                    nc.vector.tensor_reduce(
                                        out=u_w[:, kb0 * F : (kb0 + r) * F],
                                        in_=msgs[:, loc : loc + r * d, :].rearrange(
                                            "p (r d) f -> p r f d", r=r
                                        ),
                                        axis=mybir.AxisListType.X,
                                        op=mybir.AluOpType.add,
                                    )
                                    loc += r * d
                            off8 += nidx // 16
                        nc.sync.dma_start(out=u_dram[w][:, :], in_=u_w[:])
                        # align gather for this window right away: w1's
                        # align overlaps w2's message gathers
                        if stop not in ("gather", "reduce"):
                            t = bigp.tile(
                                [128, KB, F], f32, tag="gT" if w else "ual0"
                            )
                            nc.gpsimd.dma_gather(
                                out_ap=t[:],
                                in_ap=u_dram[w][:, :],
                                idxs_ap=al_sb[w][:],
                                num_idxs=SH,
                                num_idxs_reg=SH,
                                elem_size=F,
                                single_packet=False,
                                queue_num=next_q(),
                            )
                            u_al.append(t)
                    if stop in ("gather", "reduce"):
                        dbg = sb.tile([128, F], f32, tag="dbg")
                        nc.sync.dma_start(out=dbg[:], in_=u_dram[0][0:128, :])
                        nc.sync.dma_start(out=x_out[l][:, 0:F], in_=dbg[:])
                        continue

                    # ---- D: add aligned windows + self ----
                    u = u_al[0][:].rearrange("p k f -> p (k f)")
                    ual1_2d = u_al[1][:].rearrange("p k f -> p (k f)")
                    chunked = (not with_bias) and l < 2 and stop == "full"
                    if not chunked:
                        nc.vector.tensor_tensor(
                            out=u, in0=u, in1=ual1_2d, op=mybir.AluOpType.add
                        )
                        # self-loop: g_i (= dis_i * h_i) added on-chip
                        nc.vector.tensor_tensor(
                            out=u, in0=u, in1=g_nm[:], op=mybir.AluOpType.add
                        )
                    if stop == "align":
                        nc.sync.dma_start(out=x_out[l][:, :], in_=u)
                        continue

                    # ---- E: postproc ----
                    out_sb = bigp.tile([128, KB * F], f32, tag="gnm")
                    if l < 2:
                        xp = bigp.tile([128, KB * F], f32, tag="uw")
                    dis3 = dis_sb[:].unsqueeze(-1).broadcast_to([128, KB, F])
                    dis23 = dis2_sb[:].unsqueeze(-1).broadcast_to([128, KB, F])
                    u3 = u_al[0][:]
                    out3 = out_sb[:].rearrange("p (k f) -> p k f", k=KB)
                    if not with_bias:
                        # r = relu(u); out = dis*r; x' = dis2*r
                        if l < 2 and chunked:
                            # chunk adds+relu+mults+next-layer transposes so
                            # DVE/Act/PE pipeline across block ranges
                            r3 = u_al[1][:]
                            r2d = r3.rearrange("p k f -> p (k f)")
                            xp3 = xp[:].rearrange("p (k f) -> p k f", k=KB)
                            CHS = [0, 24, 48, 72, KB]
                            for ci in range(4):
                                ka, ke = CHS[ci], CHS[ci + 1]
                                sl = slice(ka * F, ke * F)
                                nc.vector.tensor_tensor(
                                    out=u[:, sl], in0=u[:, sl],
                                    in1=ual1_2d[:, sl], op=mybir.AluOpType.add,
                                )
                                nc.vector.tensor_tensor(
                                    out=u[:, sl], in0=u[:, sl],
                                    in1=g_nm[:, sl], op=mybir.AluOpType.add,
                                )
                                nc.scalar.activation(
                                    out=r2d[:, sl], in_=u[:, sl],
                                    func=mybir.ActivationFunctionType.Relu,
                                )
                                nc.vector.tensor_tensor(
                                    out=out3[:, ka:ke], in0=r3[:, ka:ke],
                                    in1=dis3[:, ka:ke], op=mybir.AluOpType.mult,
                                )
                                nc.vector.tensor_tensor(
                                    out=xp3[:, ka:ke], in0=r3[:, ka:ke],
                                    in1=dis23[:, ka:ke], op=mybir.AluOpType.mult,
                                )
                                transpose_to_xpT(xp, ka, ke)
                        elif l < 2:
                            r3 = u_al[1][:]
                            nc.scalar.activation(
                                out=r3.rearrange("p k f -> p (k f)"),
                                in_=u,
                                func=mybir.ActivationFunctionType.Relu,
                            )
                            xp3 = xp[:].rearrange("p (k f) -> p k f", k=KB)
                            nc.vector.tensor_tensor(
                                out=out3, in0=r3, in1=dis3, op=mybir.AluOpType.mult
                            )
                            nc.vector.tensor_tensor(
                                out=xp3, in0=r3, in1=dis23, op=mybir.AluOpType.mult
                            )
                        else:
                            nc.vector.tensor_tensor(
                                out=out3, in0=u3, in1=dis3, op=mybir.AluOpType.mult
                            )
                    else:
                        # v = dis*u ; t = relu(v + b) (layers 1,2) / t = v + b (layer 3)
                        # out = t ; x' = dis*t
                        v = u_al[1][:].rearrange("p k f -> p (k f)")
                        for k in range(KB):
                            s = slice(k * F, (k + 1) * F)
                            nc.scalar.mul(
                                out=v[:, s], in_=u[:, s], mul=dis_sb[:, k : k + 1]
                            )
                        bb = b_sb[l][:].to_broadcast([128, F])
                        for k in range(KB):
                            s = slice(k * F, (k + 1) * F)
                            nc.vector.tensor_tensor(
                                out=out_sb[:, s], in0=v[:, s], in1=bb, op=mybir.AluOpType.add
                            )
                        if l < 2:
                            nc.scalar.activation(
                                out=out_sb[:],
                                in_=out_sb[:],
                                func=mybir.ActivationFunctionType.Relu,
                            )
                            for k in range(KB):
                                s = slice(k * F, (k + 1) * F)
                                nc.scalar.mul(
                                    out=xp[:, s],
                                    in_=out_sb[:, s],
                                    mul=dis_sb[:, k : k + 1],
                                )
                    nc.sync.dma_start(out=x_out[l][:, :], in_=out_sb[:])

                    # ---- F: next-layer x'^T ----
                    if l < 2 and not chunked:
                        transpose_to_xpT(xp)

    nc.compile()
    return nc


# ============================ entry point =================================

def _get_compiled(edge_index, biases_zero, reps):
    key = ("k", int(np.asarray(edge_index).sum() & 0xFFFFFFF), biases_zero, reps)
    if key not in _CACHE:
        meta = _preprocess(np.asarray(edge_index))
        nc = _build(meta, with_bias=not biases_zero, reps=reps)
        _CACHE[key] = (meta, nc)
    return _CACHE[key]


def _prepare(x, edge_index, W1, b1, W2, b2, W3, b3, _reps=1):
    x = np.asarray(x, np.float32)
    biases_zero = all(
        float(np.abs(np.asarray(b)).max()) == 0.0 for b in (b1, b2, b3)
    )
    meta, nc = _get_compiled(edge_index, biases_zero, _reps)
    percore, deg_arr = meta["percore"], meta["deg_arr"]

    W3p = np.zeros((F, F), np.float32)
    W3p[:, :OC] = np.asarray(W3, np.float32)
    b3p = np.zeros((F,), np.float32)
    b3p[:OC] = np.asarray(b3, np.float32)
    Ws = [np.asarray(W1, np.float32), np.asarray(W2, np.float32), W3p]
    bs = [
        np.asarray(b1, np.float32).reshape(1, F),
        np.asarray(b2, np.float32).reshape(1, F),
        b3p.reshape(1, F),
    ]

    in_maps = []
    for c in range(NCORES):
        nodes_c = percore[c]
        x_nm = np.zeros((128, KB * F), np.float32)
        jj = np.arange(len(nodes_c))
        kk, pp = jj // 128, jj % 128
        x_nm_3d = x_nm.reshape(128, KB, F)
        x_nm_3d[pp, kk, :] = x[nodes_c]
        m = {
            "x_nm": x_nm,
            "deg": deg_arr[c],
            "w1": Ws[0], "w2": Ws[1], "w3": Ws[2],
            "b1": bs[0], "b2": bs[1], "b3": bs[2],
            "idxw1": meta["idx_blob"][0][c],
            "idxw2": meta["idx_blob"][1][c],
            "alw1": meta["al_idx"][0][c],
            "alw2": meta["al_idx"][1][c],
        }
        in_maps.append(m)

    return meta, nc, in_maps


def kernel(x, edge_index, W1, b1, W2, b2, W3, b3, _reps=1):
    meta, nc, in_maps = _prepare(x, edge_index, W1, b1, W2, b2, W3, b3, _reps)
    percore = meta["percore"]
    res = bass_utils.run_bass_kernel_spmd(nc, in_maps, core_ids=list(range(NCORES)))

    # unshard: x_out tiles [128, KB*F] -> per-node rows
    out = np.empty((N, 160), np.float32)
    for c in range(NCORES):
        nodes_c = percore[c]
        jj = np.arange(len(nodes_c))
        kk, pp = jj // 128, jj % 128
        x1 = res.results[c]["x1o"].reshape(128, KB, F)[pp, kk, :]
        x2 = res.results[c]["x2o"].reshape(128, KB, F)[pp, kk, :]
        x3 = res.results[c]["x3o"].reshape(128, KB, F)[pp, kk, :OC]
        out[nodes_c] = np.concatenate([x3, x2, x1], axis=1)
    return out

